# revision 1
# baseline (speedup 1.0000x reference)
"""Trainium2 Bass kernel for nn_DoubleNet (two GATNet branches + avg-pool + linear).

Strategy (8 NeuronCores):
  - Cores 0-3 run branch A, cores 4-7 run branch B (same SPMD program,
    different input data per core).
  - Within a branch, dst nodes are sharded contiguously across the 4 cores.
  - Per GAT layer:
      dense phase: every core computes the full z_aug = x @ [W | W@al | W@ar]
        table (z interleaved with ones-columns for the softmax denominator,
        el/er appended in the row tail) and writes it to its own DRAM; the
        er-side gather reads the 256B-aligned row tail of the same table.
      edge phase: edges are pre-sorted by dst (host side) and processed in
        chunks of 128; per chunk a dma_gather pulls z_aug[src] rows, attention
        weights w = exp(leaky_relu(el[src]+er[dst])) are computed per edge, and
        a w-scaled one-hot matmul on the PE scatter-adds messages AND the
        denominator into PSUM per 128-dst block.
      linear: x_next^T = Wl^T @ agg^T computed per block; shards are
        all-gathered across the 4 cores of the branch.
  - Final layer pools via a host-precomputed gid one-hot matmul; host sums the
    per-core partial pools and applies the output linear.
"""

import sys

sys.path.insert(0, "/opt/trn_rl_repo")

import numpy as np


# ---------------------------------------------------------------------------
# configuration
# ---------------------------------------------------------------------------

class Cfg:
    def __init__(self, N=20000, G=128, H=3, EMB=128, F=128, n_cores=8,
                 table_bf16=True, neg_slope=0.2):
        assert F == 128 and EMB == 128 and H == 3
        self.N, self.G, self.H, self.EMB, self.F = N, G, H, EMB, F
        self.n_cores = n_cores
        self.gpb = n_cores // 2            # cores per branch
        assert N % self.gpb == 0
        self.SH = N // self.gpb            # dst nodes per core
        self.NB = -(-self.SH // 128)       # dst blocks per core
        self.NT = -(-N // 128)             # node chunks for dense phase
        self.NTP = self.NT * 128           # padded node count
        self.neg_slope = neg_slope
        self.table_bf16 = table_bf16
        if table_bf16:
            self.ZC, self.EC = 512, 128    # table cols (bf16: 1024B / 256B)
        else:
            self.ZC, self.EC = 448, 64     # f32: 1792B / 256B rows
        self.GC = 8                        # chunks per z-gather call
        self.dma_scratch = 16384           # SWDGE ring carveout bytes/partition
        # z_aug column layout: z0|1|z1|1|z2|1|el  -> used cols = 390
        self.EL0 = 387                     # el columns 387:390


# ---------------------------------------------------------------------------
# host-side data prep
# ---------------------------------------------------------------------------

def _prep_edges(cfg, src, dst, q):
    """Edges of one core (dst in its shard), dst-sorted, fake rows added."""
    lo = q * cfg.SH
    sel = (dst >= lo) & (dst < lo + cfg.SH)
    es = src[sel].astype(np.int64)
    ed = (dst[sel].astype(np.int64) - lo)
    nfake = cfg.NB * 128 - cfg.SH
    if nfake:
        es = np.concatenate([es, np.zeros(nfake, np.int64)])
        ed = np.concatenate([ed, np.arange(cfg.SH, cfg.NB * 128, dtype=np.int64)])
    order = np.argsort(ed, kind="stable")
    es, ed = es[order], ed[order]
    cnt = np.bincount(ed // 128, minlength=cfg.NB)
    return es, ed, cnt, lo


def _pack_core(cfg, es, ed, lo, nc_b):
    """Build flat (block, chunk, slot) arrays padded to nc_b chunks/block."""
    TOT = int(nc_b.sum())
    zsrc = np.zeros(TOT * 128, np.int16)
    edst = np.zeros(TOT * 128, np.int16)
    dst3 = np.full(TOT * 128, -1.0, np.float32)
    epos = np.searchsorted(ed, np.arange(0, cfg.NB * 128 + 1, 128))
    c0 = 0
    for b in range(cfg.NB):
        s, e = epos[b], epos[b + 1]
        n = e - s
        o = c0 * 128
        zsrc[o:o + n] = es[s:e]
        # fake rows (local id >= SH) must not use an out-of-range er index
        ei = ed[s:e] + lo
        ei[ed[s:e] >= cfg.SH] = 0
        edst[o:o + n] = ei
        dst3[o:o + n] = (ed[s:e] - b * 128).astype(np.float32)
        c0 += nc_b[b]
    # index tiles: flat i -> (partition i%16, col i//16), replicated to 128 rows
    def wrap(a):
        return np.tile(a.reshape(-1, 16).T, (8, 1)).copy()
    # dst3 partition-major: [128, TOT]
    d3 = dst3.reshape(TOT, 128).T.copy()
    return wrap(zsrc), wrap(edst), d3


def _prep_branch_weights(cfg, W1, al1, ar1, b1, Wn, aln, arn, bn, Wl, bl):
    H, EMB = cfg.H, cfg.EMB

    def waug(W, al, ar):
        K = W.shape[0]
        out = np.zeros((K, 390), np.float32)
        out[:, :384] = W
        for h in range(H):
            out[:, 384 + h] = W[:, h * EMB:(h + 1) * EMB] @ al[h]
            out[:, 387 + h] = W[:, h * EMB:(h + 1) * EMB] @ ar[h]
        return out

    wl3 = Wl.reshape(3, 128, EMB).astype(np.float32)
    blp1 = (b1 @ Wl + bl).astype(np.float32)
    blpn = (bn @ Wl + bl).astype(np.float32)
    return waug(W1, al1, ar1), waug(Wn, aln, arn), wl3, blp1, blpn


# ---------------------------------------------------------------------------
# device program
# ---------------------------------------------------------------------------

def build_program(cfg, nc_b, timing_mode=False, skip=()):
    import concourse.bass as bass
    import concourse.mybir as mybir
    import concourse.tile as tile
    from concourse import bacc

    dt = mybir.dt
    f32 = dt.float32
    DTT = dt.bfloat16 if cfg.table_bf16 else dt.float32
    Alu = mybir.AluOpType
    Act = mybir.ActivationFunctionType

    NB, NT, SH, GC = cfg.NB, cfg.NT, cfg.SH, cfg.GC
    ZC, EC, EL0 = cfg.ZC, cfg.EC, cfg.EL0
    TOT = int(nc_b.sum())
    cum = np.concatenate([[0], np.cumsum(nc_b)]).astype(int)
    gpb = cfg.gpb
    groups = [list(range(gpb)), list(range(gpb, 2 * gpb))]

    nc = bacc.Bacc("TRN2", target_bir_lowering=False, debug=False,
                   num_devices=cfg.n_cores,
                   dynamic_dma_scratch_size=cfg.dma_scratch)

    # inputs -----------------------------------------------------------------
    xT0 = nc.dram_tensor("xT0", [128, cfg.NTP], f32, kind="ExternalInput")
    waug1_d = nc.dram_tensor("waug1", [128, 390], f32, kind="ExternalInput")
    waugn_d = nc.dram_tensor("waugn", [128, 390], f32, kind="ExternalInput")
    wl3_d = nc.dram_tensor("wl3", [3, 128, 128], f32, kind="ExternalInput")
    blp1_d = nc.dram_tensor("blp1", [128, 1], f32, kind="ExternalInput")
    blpn_d = nc.dram_tensor("blpn", [128, 1], f32, kind="ExternalInput")
    iota_d = nc.dram_tensor("iota", [128, 128], DTT, kind="ExternalInput")
    ident_d = nc.dram_tensor("ident", [128, 128], f32, kind="ExternalInput")
    dst3_d = nc.dram_tensor("dst3", [128, TOT], f32, kind="ExternalInput")
    zidx_d = nc.dram_tensor("zidx", [128, TOT * 8], dt.int16, kind="ExternalInput")
    eidx_d = nc.dram_tensor("eidx", [128, TOT * 8], dt.int16, kind="ExternalInput")
    poolw_d = nc.dram_tensor("poolw", [NB, 128, 128], f32, kind="ExternalInput")
    pool_out = nc.dram_tensor("pool_out", [128, 128], f32, kind="ExternalOutput")

    # internal DRAM ----------------------------------------------------------
    zaug = nc.dram_tensor("zaug", [cfg.NTP, ZC], DTT)
    HB = NB // 2
    SH1 = min(HB * 128, SH)
    SH2 = SH - SH1
    xsh1 = nc.dram_tensor("xsh1", [128, SH1], f32)
    xsh2 = nc.dram_tensor("xsh2", [128, SH2], f32)
    xgath1 = nc.dram_tensor("xgath1", [gpb, 128, SH1], f32)
    xgath2 = nc.dram_tensor("xgath2", [gpb, 128, SH2], f32)

    def do_gather(nc, which, timing_mode):
        xs, xg = (xsh1, xgath1) if which == 0 else (xsh2, xgath2)
        if timing_mode:
            for j in range(gpb):
                nc.sync.dma_start(xg.ap()[j], xs.ap())
        else:
            nc.gpsimd.collective_compute(
                "AllGather", mybir.AluOpType.bypass, replica_groups=groups,
                ins=[xs.ap()], outs=[xg.ap()])

    with tile.TileContext(nc) as tc:
        cpool = tc.alloc_tile_pool(name="const", bufs=1)
        # persistent SBUF state
        xT = cpool.tile([128, cfg.NTP], f32, tag="xT")
        waug1 = cpool.tile([128, 390], f32, tag="waug1")
        waugn = cpool.tile([128, 390], f32, tag="waugn")
        wl3 = cpool.tile([128, 3, 128], f32, tag="wl3")
        blp1 = cpool.tile([128, 1], f32, tag="blp1")
        iota = cpool.tile([128, 128], DTT, tag="iota")
        ident = cpool.tile([128, 128], f32, tag="ident")
        dst3 = cpool.tile([128, TOT], f32, tag="dst3")
        zidx = cpool.tile([128, TOT * 8], dt.int16, tag="zidx")
        eidx = cpool.tile([128, TOT * 8], dt.int16, tag="eidx")

        nc.sync.dma_start(xT[:], xT0.ap())
        nc.sync.dma_start(waug1[:], waug1_d.ap())
        nc.sync.dma_start(waugn[:], waugn_d.ap())
        nc.sync.dma_start(wl3[:], wl3_d.ap().rearrange("k p m -> p k m"))
        nc.sync.dma_start(blp1[:], blp1_d.ap())
        nc.sync.dma_start(iota[:], iota_d.ap())
        nc.sync.dma_start(ident[:], ident_d.ap())
        nc.sync.dma_start(dst3[:], dst3_d.ap())
        nc.sync.dma_start(zidx[:], zidx_d.ap())
        nc.sync.dma_start(eidx[:], eidx_d.ap())

        psz_pool = tc.alloc_tile_pool(name="psz", bufs=3, space="PSUM")
        zst_pool = tc.alloc_tile_pool(name="zst", bufs=10)
        g_pool = tc.alloc_tile_pool(name="g", bufs=6)
        r_pool = tc.alloc_tile_pool(name="r", bufs=2)
        w_pool = tc.alloc_tile_pool(name="w", bufs=2)
        l_pool = tc.alloc_tile_pool(name="l", bufs=12)
        psb_pool = tc.alloc_tile_pool(name="psb", bufs=2, space="PSUM")
        pst_pool = tc.alloc_tile_pool(name="pst", bufs=1, space="PSUM")
        psx_pool = tc.alloc_tile_pool(name="psx", bufs=1, space="PSUM")
        s_pool = tc.alloc_tile_pool(name="s", bufs=2)
        a_pool = tc.alloc_tile_pool(name="a", bufs=2)
        at_pool = tc.alloc_tile_pool(name="at", bufs=2)
        x_pool = tc.alloc_tile_pool(name="x", bufs=2)
        pw_pool = tc.alloc_tile_pool(name="pw", bufs=2)
        pp_pool = tc.alloc_tile_pool(name="pp", bufs=1, space="PSUM")

        ps_pool_acc = pp_pool.tile([128, 128], f32, tag="poolacc")

        for layer in range(3):
            wa = waug1 if layer == 0 else waugn
            # ---------------- dense phase: z_aug + er tables ----------------
            for t in range(NT):
                psz = psz_pool.tile([128, 390], f32, tag="psz")
                nc.tensor.matmul(psz[:], xT[:, t * 128:(t + 1) * 128], wa[:],
                                 start=True, stop=True)
                zt = zst_pool.tile([128, ZC], DTT, tag="zt")
                if "stage" not in skip:
                    nc.vector.memset(zt[:, 128:388:129], 1.0)
                    nc.vector.tensor_copy(zt[:, 0:128], psz[:, 0:128])
                    nc.vector.tensor_copy(zt[:, 129:257], psz[:, 128:256])
                    nc.vector.tensor_copy(zt[:, 258:386], psz[:, 256:384])
                    nc.vector.tensor_copy(zt[:, 387:393], psz[:, 384:390])
                nc.sync.dma_start(
                    zaug.ap()[t * 128:(t + 1) * 128, 0:393], zt[:, 0:393])

            # ---------------- edge phase ------------------------------------
            for b in range(NB):
                ncb = int(nc_b[b])
                c0 = int(cum[b])
                R = r_pool.tile([128, int(nc_b.max()), EC], DTT, tag="R")
                psb = psb_pool.tile([128, 387], f32, tag="psb")
                for g0 in range(0, ncb, GC):
                    gsz = min(GC, ncb - g0)
                    Gt = g_pool.tile([128, GC, ZC], DTT, tag="G")
                    if "gather" not in skip: nc.gpsimd.dma_gather(
                        Gt[:, 0:gsz, :], zaug.ap(),
                        zidx[:, 8 * (c0 + g0): 8 * (c0 + g0 + gsz)],
                        num_idxs=gsz * 128, num_idxs_reg=gsz * 128,
                        elem_size=ZC, elem_step=ZC)
                    nc.gpsimd.dma_gather(
                        R[:, g0:g0 + gsz, :], zaug.ap()[:, 384:384 + EC],
                        eidx[:, 8 * (c0 + g0): 8 * (c0 + g0 + gsz)],
                        num_idxs=gsz * 128, num_idxs_reg=gsz * 128,
                        elem_size=EC, elem_step=ZC)
                    wt = w_pool.tile([128, GC, 3], f32, tag="wt")
                    nc.vector.tensor_tensor(
                        wt[:, 0:gsz, :], Gt[:, 0:gsz, EL0:EL0 + 3],
                        R[:, g0:g0 + gsz, 6:9], Alu.add)
                    nc.vector.scalar_tensor_tensor(
                        wt[:, 0:gsz, :], wt[:, 0:gsz, :], cfg.neg_slope,
                        wt[:, 0:gsz, :], Alu.mult, Alu.max)
                    nc.scalar.activation(wt[:, 0:gsz, :], wt[:, 0:gsz, :],
                                         Act.Exp)
                    for c in range(gsz):
                        cc = c0 + g0 + c
                        for h in range(3):
                            lh = l_pool.tile([128, 128], DTT, tag="lh")
                            if "onehot" not in skip: nc.vector.tensor_scalar(
                                lh[:], iota[:],
                                dst3[:, cc:cc + 1],
                                wt[:, c, h:h + 1].opt(),
                                Alu.is_equal, Alu.mult)
                            if "emm" in skip: continue
                            nc.tensor.matmul(
                                psb[:, 129 * h:129 * h + 129], lh[:],
                                Gt[:, c, 129 * h:129 * h + 129].opt(),
                                start=(g0 + c == 0 and h == 0),
                                stop=(g0 + c == ncb - 1 and h == 2))
                # normalize + transpose + linear
                r3 = s_pool.tile([128, 3], f32, tag="r3")
                nc.vector.reciprocal(r3[:], psb[:, 128:387:129])
                agg = a_pool.tile([128, 384], f32, tag="agg")
                for h in range(3):
                    nc.vector.tensor_scalar(
                        agg[:, 128 * h:128 * (h + 1)],
                        psb[:, 129 * h:129 * h + 128],
                        r3[:, h:h + 1], None, Alu.mult)
                aggT = at_pool.tile([128, 3, 128], f32, tag="aggT")
                for k in range(3):
                    pst = pst_pool.tile([128, 128], f32, tag="pst")
                    nc.tensor.transpose(pst[:], agg[:, 128 * k:128 * (k + 1)],
                                        ident[:])
                    nc.vector.tensor_copy(aggT[:, k, :].opt(), pst[:])
                bw = min(128, SH - b * 128)
                if layer < 2:
                    psx = psx_pool.tile([128, 128], f32, tag="psx")
                    for k in range(3):
                        nc.tensor.matmul(psx[:], wl3[:, k, :].opt(),
                                         aggT[:, k, :].opt(),
                                         start=(k == 0), stop=(k == 2))
                    xsb = x_pool.tile([128, 128], f32, tag="xsb")
                    nc.vector.tensor_scalar(xsb[:], psx[:], blp1[:], None,
                                            Alu.add)
                    if b < HB:
                        nc.sync.dma_start(
                            xsh1.ap()[:, b * 128:b * 128 + bw], xsb[:, 0:bw])
                    else:
                        o = b * 128 - SH1
                        nc.sync.dma_start(
                            xsh2.ap()[:, o:o + bw], xsb[:, 0:bw])
                    if b == HB - 1:
                        do_gather(nc, 0, timing_mode)
                else:
                    psx = psx_pool.tile([128, 128], f32, tag="psx")
                    for k in range(3):
                        nc.tensor.matmul(psx[:], aggT[:, k, :].opt(),
                                         wl3[:, k, :].opt(),
                                         start=(k == 0), stop=(k == 2))
                    x3 = x_pool.tile([128, 128], f32, tag="xsb")
                    nc.vector.tensor_copy(x3[:], psx[:])
                    pw = pw_pool.tile([128, 128], f32, tag="pw")
                    nc.sync.dma_start(pw[:], poolw_d.ap()[b])
                    nc.tensor.matmul(ps_pool_acc[:], pw[:], x3[:],
                                     start=(b == 0), stop=(b == NB - 1))

            if layer < 2:
                do_gather(nc, 1, timing_mode)
                for j in range(gpb):
                    nc.sync.dma_start(xT[:, j * SH:j * SH + SH1],
                                      xgath1.ap()[j])
                    nc.sync.dma_start(xT[:, j * SH + SH1:(j + 1) * SH],
                                      xgath2.ap()[j])
                if layer == 0:
                    nc.sync.dma_start(blp1[:], blpn_d.ap())

        po = x_pool.tile([128, 128], f32, tag="po")
        nc.vector.tensor_copy(po[:], ps_pool_acc[:])
        nc.sync.dma_start(pool_out.ap(), po[:])

        for p in (pp_pool, pw_pool, x_pool, at_pool, a_pool, s_pool,
                  psx_pool, pst_pool, psb_pool, l_pool, w_pool, r_pool,
                  g_pool, zst_pool, psz_pool, cpool):
            p.release()

    nc.compile()
    return nc


# ---------------------------------------------------------------------------
# top-level kernel
# ---------------------------------------------------------------------------

def _prepare(cfg, inputs):
    """Returns (nc_b, in_maps, host_meta)."""
    npf = np.asarray
    per_core_edges = []
    nc_b = np.zeros(cfg.NB, np.int64)
    for br, (s, d) in enumerate((("srcA", "dstA"), ("srcB", "dstB"))):
        src = npf(inputs[s]).astype(np.int64)
        dst = npf(inputs[d]).astype(np.int64)
        for q in range(cfg.gpb):
            es, ed, cnt, lo = _prep_edges(cfg, src, dst, q)
            per_core_edges.append((es, ed, lo))
            nc_b = np.maximum(nc_b, -(-cnt // 128))
    in_maps = []
    host_meta = {}
    if cfg.table_bf16:
        import ml_dtypes
        tdt = ml_dtypes.bfloat16
    else:
        tdt = np.float32
    iota = np.tile(np.arange(128, dtype=tdt), (128, 1))
    ident = np.eye(128, dtype=np.float32)
    for br in range(2):
        sfx = "AB"[br]
        W1 = npf(inputs["W1" + sfx]); al1 = npf(inputs["al1" + sfx])
        ar1 = npf(inputs["ar1" + sfx]); b1 = npf(inputs["b1" + sfx])
        Wn = npf(inputs["Wn" + sfx]); aln = npf(inputs["aln" + sfx])
        arn = npf(inputs["arn" + sfx]); bn = npf(inputs["bn" + sfx])
        Wl = npf(inputs["Wl" + sfx]); bl = npf(inputs["bl" + sfx])
        gid = npf(inputs["gid" + sfx]).astype(np.int64)
        feats = npf(inputs["feats" + sfx]).astype(np.float32)
        waug1, waugn, wl3, blp1, blpn = _prep_branch_weights(
            cfg, W1, al1, ar1, b1, Wn, aln, arn, bn, Wl, bl)
        xT0 = np.zeros((128, cfg.NTP), np.float32)
        xT0[:, :cfg.N] = feats.T
        host_meta[sfx] = dict(blpn=blpn, gid=gid)
        for q in range(cfg.gpb):
            es, ed, lo = per_core_edges[br * cfg.gpb + q]
            zidx, eidx, dst3 = _pack_core(cfg, es, ed, lo, nc_b)
            poolw = np.zeros((cfg.NB, 128, 128), np.float32)
            for b in range(cfg.NB):
                for i in range(min(128, cfg.SH - b * 128)):
                    n = lo + b * 128 + i
                    if n < cfg.N:
                        poolw[b, i, gid[n]] = 1.0
            in_maps.append({
                "xT0": xT0, "waug1": waug1, "waugn": waugn,
                "wl3": wl3, "blp1": blp1.reshape(128, 1),
                "blpn": blpn.reshape(128, 1),
                "iota": iota, "ident": ident,
                "dst3": dst3, "zidx": zidx, "eidx": eidx, "poolw": poolw,
            })
    return nc_b, in_maps, host_meta


def _finalize(cfg, inputs, host_meta, pool_outs):
    """pool_outs: list of 8 [128,128] arrays -> full output [G,1] float64."""
    out = {}
    for br in range(2):
        sfx = "AB"[br]
        total = np.zeros((128, 128), np.float64)
        for q in range(cfg.gpb):
            total += pool_outs[br * cfg.gpb + q].astype(np.float64)
        gid = host_meta[sfx]["gid"]
        cnt = np.bincount(gid, minlength=128).astype(np.float64)
        total += cnt[:, None] * host_meta[sfx]["blpn"].astype(np.float64)[None, :]
        out[sfx] = (total / np.maximum(cnt[:, None], 1.0))[:cfg.G]
    cat = np.concatenate([out["A"], out["B"]], axis=1)
    Wo = np.asarray(inputs["Wo"]).astype(np.float64)
    bo = np.asarray(inputs["bo"]).astype(np.float64)
    return (cat @ Wo + bo).astype(np.float64)


_CACHE = {}


def kernel(**inputs):
    cfg = Cfg(N=inputs["featsA"].shape[0], G=128)
    nc_b, in_maps, host_meta = _prepare(cfg, inputs)
    key = ("prog", tuple(nc_b.tolist()), cfg.table_bf16)
    if key not in _CACHE:
        _CACHE[key] = build_program(cfg, nc_b)
    nc = _CACHE[key]
    from concourse.bass_utils import run_bass_kernel_spmd
    res = run_bass_kernel_spmd(nc, in_maps, list(range(cfg.n_cores)))
    pool_outs = [r["pool_out"] for r in res.results]
    return _finalize(cfg, inputs, host_meta, pool_outs)

